# revision 39
# baseline (speedup 1.0000x reference)
"""AffineLabelAttention Trainium2 kernel (v5).

out[b, l, i, j] = W_h[l] @ head[b, i] + W_d[l] @ dep[b, j] + bias[l]

Shapes (hardcoded): head/dep [4, 1024, 768] f32, label_W [32, 1536], label_b [32].
Full output [4, 32, 1024, 1024] f32 (512 MB) -> completely output-DMA-bound.

Sharding over 8 cores: core c handles batch b = c // 2 and label half
lh = c % 2 (16 labels).

The device stores the output in float16 (pointwise rel err <= 2^-11);
the host upcasts during the unshard. 32 MB of output per core is the
whole cost: a single HWDGE queue fans each DMA across all 16 SDMA
engines and sustains ~420 GB/s (measured), so one queue IS the
roofline. The kernel's only job is to start that stream as early as
possible and never let it starve.

Structure (what profiling showed matters):
  1. Inputs host-cast to f16 and host-rearranged to a per-partition
     contiguous [p][jc][k][s] layout; every PE matmul is 1-pass f16.
  2. NO SWDGE (gpsimd) DMAs: the Q7 software descriptor generation for
     small constants sprays hundreds of tiny ring descriptors whose
     SBUF-port traffic stalls the HWDGE SDMA engines for ~6 us right
     in the middle of input staging. All constants (weights, one-hot
     selectors, transpose identity, bias) are packed into ONE
     zero-padded [128, 2304] f16 tensor, the sync ring's first
     transfer (~0.6 us).
  3. Staging order: consts, then dep (both HWDGE rings), then head
     j-half 0, then j-half 1. The PE warm-up chain is sized to end
     right as dep lands: the HAM clock boost needs ~3-4 us of
     CONTINUOUS PE duty, and any idle gap before the score matmuls
     drops the whole score phase to half clock (~8 us of extra start
     latency).
  4. Per label: d-row broadcast via one-hot PE matmuls into PSUM,
     evacuated by ACT to persistent f16 SBUF tiles; adds run 6 on DVE
     (f16 SBUF tensor_scalar = 4x perf mode, ~480 ns per [128,1024]
     tile) and 2 on ACT. Both engines finish just under the 2 MB DMA
     drain time, so trigger admission tracks the drain rate and the
     16 SDMA engines stay in lockstep (bursty admission lets the slow
     engine 15 build a private backlog that drains alone as a 6-13 us
     tail after the last trigger).
  5. Label 0 is split 1 MB + 1 MB with its first-group adds split
     DVE/ACT so the stream launches before the h j-half-1 path
     (matmul + transposes) resolves.

  Notes baked into the structure:
  - walrus/bass: compute-engine operands must start at partition
    0/32/64 (96 is rejected); engines cannot move data across
    partitions (only PE matmul/transpose and DMA can).
  - PSUM is 8 banks x 2KB: score pool 2 + warmup/transpose pool 2 +
    broadcast pool 4.
  - A DMA trigger that cannot get a ring slot stalls its issuing
    engine: ACT issues only 3 input transfers, all before its compute.
  - PSUM operands cap DVE perf modes, so broadcasts are evacuated to
    SBUF f16 by ACT (closest to PSUM) and the adds read SBUF at 4x.
"""

import sys

import numpy as np

if "/opt/trn_rl_repo" not in sys.path:
    sys.path.insert(0, "/opt/trn_rl_repo")

import concourse.bass as bass
import concourse.mybir as mybir
from concourse import bacc
from concourse.bass_utils import run_bass_kernel_spmd
from concourse.tile import TileContext, add_dep_helper

B, S, D, L = 4, 1024, 768, 32
NCORES = 8
LH = L // 2          # labels per core (16)
KCH = D // 128       # contraction chunks (6)
ICH = S // 128       # i chunks (8)
F32 = mybir.dt.float32
F16 = mybir.dt.float16
WU_N = 9             # PE warm-up matmuls before the score streams

# packed layouts (f16)
# pkD [128, 6336]: weights cols k*32 + (0:16)=W_h, (16:32)=W_d, then
#                  dep at cols 192 + jc*3072 + k*512 + s'
# pkS [80, 2080]:  one-hot selectors rows 0:48 cols 0:2048; transpose
#                  identity rows 64:80 cols 2048:2064; bias col 2064
PKA_D = 192          # dep column base in pkD
PKA_N = PKA_D + 2 * KCH * 512
PKS_ID = LH * 128    # 2048
PKS_B = PKS_ID + LH  # 2064
PKS_N = 2080

# knobs for test harness
TRACE = False
TRACE_CORES = None
LAST_RESULTS = None

_CACHE = {}


def _build():
    nc = bacc.Bacc("TRN2", target_bir_lowering=False, debug=False)
    # inputs pre-rearranged on host: [partition, jc, k, s'] where
    # d = k*128 + p contracts and j (or i) = jc*512 + s'
    headT = nc.dram_tensor("headT", [128, 2, KCH, 512], F16,
                           kind="ExternalInput")
    pkdd = nc.dram_tensor("pkD", [128, PKA_N], F16, kind="ExternalInput")
    pksd = nc.dram_tensor("pkS", [80, PKS_N], F16, kind="ExternalInput")
    # [l, p, c, j]: row i = c*128 + p of label l lives at out[l, p, c, :]
    out = nc.dram_tensor("out", [LH, 128, ICH, S], F16, kind="ExternalOutput")
    out_v = out[:]

    headT_f = headT[:]

    with TileContext(nc) as tc:
        with (
            tc.tile_pool(name="const", bufs=1) as cpool,
            tc.tile_pool(name="outp", bufs=4) as opool,
            tc.tile_pool(name="bcast", bufs=16) as bpool,
            tc.tile_pool(name="psum_sc", bufs=2, space="PSUM") as psc,
            tc.tile_pool(name="psum_tp", bufs=2, space="PSUM") as ptp,
            tc.tile_pool(name="psum_bc", bufs=4, space="PSUM") as pbc,
        ):
            pk_d = cpool.tile([128, PKA_N], F16)
            headT_sb = cpool.tile([128, 2, KCH, 512], F16)
            pk_sel = cpool.tile([80, PKS_N], F16)
            h_lT = cpool.tile([128, S], F16)     # h scores [l, i] @ parts 64:80
            h_all = cpool.tile([128, ICH, LH], F32)  # h scores, [i, l] layout
            d_sb = cpool.tile([48, S], F16)      # d+bias: jc0 @ 0:16, jc1 @ 32:48
            wu_w = cpool.tile([128, LH], F16)    # PE warm-up operands
            wu_x = cpool.tile([128, 512], F16)

            def w_h(k):
                return pk_d[:, k * 32:k * 32 + LH]

            def w_d(k):
                return pk_d[:, k * 32 + LH:k * 32 + 2 * LH]

            def dep_v(jc, k):
                c0 = PKA_D + jc * KCH * 512 + k * 512
                return pk_d[:, c0:c0 + 512]

            def sel_v(jc, lb):
                p0 = 32 * jc
                return pk_sel[p0:p0 + LH, lb * 128:(lb + 1) * 128]

            id_v = pk_sel[64:64 + LH, PKS_ID:PKS_ID + LH]
            b_col = pk_sel[0:48, PKS_B:PKS_B + 1]

            # Warm-up operand memsets first so DVE clears them at t~0 and
            # the PE warm-up chain starts immediately.
            nc.vector.memset(wu_w[:], 0.0)
            nc.vector.memset(wu_x[:], 0.0)

            # --- input staging -------------------------------------------
            # HWDGE descriptor generation is ONE shared TPB-level engine:
            # every transfer's generation serializes (~2 us each) no
            # matter which ring it sits on, and a single queue's drain
            # already hits ~420 GB/s. So: exactly TWO input transfers,
            # both on the sync ring (weights+dep pack first, full head
            # second - each gen hides under the previous drain), and the
            # scalar ring stays empty so ACT goes straight to compute.
            # The selector pack rides SWDGE as ONE medium transfer (many
            # small SWDGE transfers spray descriptor-ring traffic that
            # stalls the HWDGE SDMA engines).
            nc.sync.dma_start(out=pk_d[:], in_=pkdd[:])
            nc.gpsimd.dma_start(out=pk_sel[:], in_=pksd[:])
            nc.sync.dma_start(out=headT_sb[:], in_=headT_f[:])

            # PE warm-up (builds HAM clock duty while inputs stream in;
            # sized to end about when dep lands). Lives in the transpose
            # pool so it never blocks the score psums.
            wu_ps = ptp.tile([128, 512], F32, name="wu", tag="tp")
            for _ in range(WU_N):
                nc.tensor.matmul(wu_ps[0:LH, :], wu_w[:], wu_x[:],
                                 start=True, stop=True)

            # d scores: two concurrent column-group streams (jc0 @ group 0,
            # jc1 @ group 32), issue-interleaved so the array pipelines the
            # LdWeights of one group under the matmul of the other.
            sc_a = psc.tile([128, 512], F32, name="sc_a", tag="score")
            sc_b = psc.tile([128, 512], F32, name="sc_b", tag="score")
            for k in range(KCH):
                nc.tensor.matmul(
                    sc_a[0:LH, :], w_d(k), dep_v(0, k),
                    start=(k == 0), stop=(k == KCH - 1),
                    tile_position=(0, 0),
                )
                nc.tensor.matmul(
                    sc_b[32:32 + LH, :], w_d(k), dep_v(1, k),
                    start=(k == 0), stop=(k == KCH - 1),
                    tile_position=(0, 32),
                )

            # d evacuation (+bias) on ACT (fastest PSUM reader), f16 out
            nc.scalar.add(d_sb[0:LH, 0:512], sc_a[0:LH, :], b_col[0:LH, :])
            nc.scalar.add(d_sb[32:32 + LH, 512:1024],
                          sc_b[32:32 + LH, :], b_col[32:32 + LH, :])

            # h j-half 0 @ group 64 (needs head-jc0 only)
            sc_c = psc.tile([128, 512], F32, name="sc_c", tag="score")
            for k in range(KCH):
                nc.tensor.matmul(
                    sc_c[64:64 + LH, :], w_h(k), headT_sb[:, 0, k, :],
                    start=(k == 0), stop=(k == KCH - 1),
                    tile_position=(0, 64),
                )
            nc.scalar.copy(h_lT[64:64 + LH, 0:512], sc_c[64:64 + LH, :])

            dbcs = {}

            def bcast(lb):
                # replicate d row lb across 128 partitions: one-hot selector
                # matmuls (f16 exact). Result evacuated to a persistent f16
                # SBUF tile so the adds run in DVE 4x mode.
                dbc = bpool.tile([128, S], F16, name="dbc", tag="dbc")
                for jc in range(2):
                    bc_ps = pbc.tile([128, 512], F32, name="bc", tag="bc")
                    nc.tensor.matmul(
                        bc_ps[:], sel_v(jc, lb),
                        d_sb[32 * jc:32 * jc + LH,
                             jc * 512:(jc + 1) * 512],
                        start=True, stop=True,
                    )
                    nc.scalar.copy(dbc[:, jc * 512:(jc + 1) * 512], bc_ps[:])
                dbcs[lb] = dbc

            # first broadcast as soon as d_sb exists (bcast(1) comes after
            # label 0's first adds so it never sits ahead of them in the
            # in-order ACT queue)
            bcast(0)

            # h -> [i, l] layout via PE transposes of [16, 128] blocks
            def h_transpose(ic):
                loc = ic * 128
                tp = ptp.tile([128, LH], F16, name="tp", tag="tp")
                nc.tensor.transpose(
                    tp[:], h_lT[64:64 + LH, loc:loc + 128], id_v)
                nc.vector.tensor_copy(out=h_all[:, ic, :], in_=tp[:])

            for ic in range(4):
                h_transpose(ic)

            def add_one(ot, lb, ic, on_dve):
                scal = h_all[:, ic, lb:lb + 1]
                if on_dve:
                    nc.vector.tensor_scalar_add(ot[:, ic, :], dbcs[lb][:],
                                                scal)
                else:
                    nc.scalar.add(ot[:, ic, :], dbcs[lb][:], scal)

            # label 0, first half: launches the output stream before the
            # h j-half-1 path resolves (all-DVE: ACT is still busy with
            # the dbc0 evacuation at this point)
            ot0 = opool.tile([128, ICH, S], F16, name="ot", tag="ot")
            for ic in range(4):
                add_one(ot0, 0, ic, on_dve=True)
            nc.sync.dma_start(out=out_v[0, :, 0:4, :], in_=ot0[:, 0:4, :])
            bcast(1)

            # h j-half 1, also @ group 64 (its bank is sc_b's, its column
            # group reopens once sc_c is evacuated; PE is free by then)
            sc_d = psc.tile([128, 512], F32, name="sc_d", tag="score")
            for k in range(KCH):
                nc.tensor.matmul(
                    sc_d[64:64 + LH, :], w_h(k), headT_sb[:, 1, k, :],
                    start=(k == 0), stop=(k == KCH - 1),
                    tile_position=(0, 64),
                )
            nc.scalar.copy(h_lT[64:64 + LH, 512:1024], sc_d[64:64 + LH, :])
            for ic in range(4, ICH):
                h_transpose(ic)

            # label 0, second half
            for ic in range(4, ICH):
                add_one(ot0, 0, ic, on_dve=(ic < 6))
            nc.sync.dma_start(out=out_v[0, :, 4:8, :], in_=ot0[:, 4:8, :])
            bcast(2)

            # --- steady output loop --------------------------------------
            for lb in range(1, LH):
                ot = opool.tile([128, ICH, S], F16, name="ot", tag="ot")
                for ic in range(ICH):
                    add_one(ot, lb, ic, on_dve=(ic < 6))
                nc.sync.dma_start(out=out_v[lb, :, :, :], in_=ot[:])
                # broadcasts emitted AFTER each label's adds: on the
                # in-order ACT queue the dbc copies must sit behind this
                # label's adds, or every label gates on the next label's
                # broadcast evacuation
                if lb + 2 < LH:
                    bcast(lb + 2)
    nc.compile()
    return nc


def kernel(head, dep, label_W, label_b):
    global LAST_RESULTS
    head = np.asarray(head, dtype=np.float32)
    dep = np.asarray(dep, dtype=np.float32)
    label_W = np.asarray(label_W, dtype=np.float32)
    label_b = np.asarray(label_b, dtype=np.float32)

    def pack_inp(x):  # [S, D] f32 -> [128, 2, KCH, 512] f16, d = k*128+p
        xT = np.ascontiguousarray(x.T).astype(np.float16)   # [D, S]
        return np.ascontiguousarray(
            xT.reshape(KCH, 128, 2, 512).transpose(1, 2, 0, 3))

    headP = [pack_inp(head[b]) for b in range(B)]
    depP = [pack_inp(dep[b]) for b in range(B)]
    whT = label_W[:, :D].T.astype(np.float16)   # [D, L]
    wdT = label_W[:, D:].T.astype(np.float16)   # [D, L]

    in_maps = []
    for c in range(NCORES):
        b, lh = divmod(c, 2)
        ls = slice(lh * LH, (lh + 1) * LH)
        pack_d = np.zeros((128, PKA_N), dtype=np.float16)
        # weights: pack_d[p, k*32 + c] = W[d = k*128 + p, label c]
        for k in range(KCH):
            pack_d[:, k * 32:k * 32 + LH] = whT[k * 128:(k + 1) * 128, ls]
            pack_d[:, k * 32 + LH:k * 32 + 2 * LH] = \
                wdT[k * 128:(k + 1) * 128, ls]
        # dep at cols 192+: [p, jc, k, s']
        pack_d[:, PKA_D:] = depP[b].reshape(128, 2 * KCH * 512)
        pack_s = np.zeros((80, PKS_N), dtype=np.float16)
        # one-hot selectors at partition groups 0 and 32
        for lb in range(LH):
            pack_s[lb, lb * 128:(lb + 1) * 128] = 1.0
            pack_s[32 + lb, lb * 128:(lb + 1) * 128] = 1.0
        # transpose identity at partition group 64
        pack_s[64:64 + LH, PKS_ID:PKS_ID + LH] = np.eye(LH, dtype=np.float16)
        # bias column at partition groups 0 and 32
        pack_s[0:LH, PKS_B] = label_b[ls]
        pack_s[32:48, PKS_B] = label_b[ls]
        in_maps.append({
            "headT": headP[b],
            "pkD": pack_d,
            "pkS": pack_s,
        })

    if "nc" not in _CACHE:
        _CACHE["nc"] = _build()
    nc = _CACHE["nc"]

    res = run_bass_kernel_spmd(nc, in_maps, core_ids=list(range(NCORES)),
                               trace=TRACE, trace_cores=TRACE_CORES)
    LAST_RESULTS = res

    out = np.empty((B, L, S, S), dtype=np.float32)
    for c in range(NCORES):
        b, lh = divmod(c, 2)
        # device layout [l, p, c, j] with i = c*128 + p -> [l, i, j]
        o = np.asarray(res.results[c]["out"])  # [16, 128, 8, 1024] f16
        o = o.transpose(0, 2, 1, 3).reshape(LH, S, S)
        out[b, lh * LH:(lh + 1) * LH] = o.astype(np.float32)
    return out


# revision 43
# speedup vs baseline: 1.0591x; 1.0591x over previous
"""AffineLabelAttention Trainium2 kernel (v5).

out[b, l, i, j] = W_h[l] @ head[b, i] + W_d[l] @ dep[b, j] + bias[l]

Shapes (hardcoded): head/dep [4, 1024, 768] f32, label_W [32, 1536], label_b [32].
Full output [4, 32, 1024, 1024] f32 (512 MB) -> completely output-DMA-bound.

Sharding over 8 cores: core c handles batch b = c // 2 and label half
lh = c % 2 (16 labels).

The device stores the output in float16 (pointwise rel err <= 2^-11);
the host upcasts during the unshard. 32 MB of output per core is the
whole cost: a single HWDGE queue fans each DMA across all 16 SDMA
engines and sustains ~420 GB/s (measured), so one queue IS the
roofline. The kernel's only job is to start that stream as early as
possible and never let it starve.

Structure (what profiling showed matters):
  1. Inputs host-cast to f16 and host-rearranged to a per-partition
     contiguous [p][jc][k][s] layout; every PE matmul is 1-pass f16.
  2. NO SWDGE (gpsimd) DMAs: the Q7 software descriptor generation for
     small constants sprays hundreds of tiny ring descriptors whose
     SBUF-port traffic stalls the HWDGE SDMA engines for ~6 us right
     in the middle of input staging. All constants (weights, one-hot
     selectors, transpose identity, bias) are packed into ONE
     zero-padded [128, 2304] f16 tensor, the sync ring's first
     transfer (~0.6 us).
  3. Staging order: consts, then dep (both HWDGE rings), then head
     j-half 0, then j-half 1. The PE warm-up chain is sized to end
     right as dep lands: the HAM clock boost needs ~3-4 us of
     CONTINUOUS PE duty, and any idle gap before the score matmuls
     drops the whole score phase to half clock (~8 us of extra start
     latency).
  4. Per label: d-row broadcast via one-hot PE matmuls into PSUM,
     evacuated by ACT to persistent f16 SBUF tiles; adds run 6 on DVE
     (f16 SBUF tensor_scalar = 4x perf mode, ~480 ns per [128,1024]
     tile) and 2 on ACT. Both engines finish just under the 2 MB DMA
     drain time, so trigger admission tracks the drain rate and the
     16 SDMA engines stay in lockstep (bursty admission lets the slow
     engine 15 build a private backlog that drains alone as a 6-13 us
     tail after the last trigger).
  5. Label 0 is split 1 MB + 1 MB with its first-group adds split
     DVE/ACT so the stream launches before the h j-half-1 path
     (matmul + transposes) resolves.

  Notes baked into the structure:
  - walrus/bass: compute-engine operands must start at partition
    0/32/64 (96 is rejected); engines cannot move data across
    partitions (only PE matmul/transpose and DMA can).
  - PSUM is 8 banks x 2KB: score pool 2 + warmup/transpose pool 2 +
    broadcast pool 4.
  - A DMA trigger that cannot get a ring slot stalls its issuing
    engine: ACT issues only 3 input transfers, all before its compute.
  - PSUM operands cap DVE perf modes, so broadcasts are evacuated to
    SBUF f16 by ACT (closest to PSUM) and the adds read SBUF at 4x.
"""

import sys

import numpy as np

if "/opt/trn_rl_repo" not in sys.path:
    sys.path.insert(0, "/opt/trn_rl_repo")

import concourse.bass as bass
import concourse.mybir as mybir
from concourse import bacc
from concourse.bass_utils import run_bass_kernel_spmd
from concourse.tile import TileContext, add_dep_helper

B, S, D, L = 4, 1024, 768, 32
NCORES = 8
LH = L // 2          # labels per core (16)
KCH = D // 128       # contraction chunks (6)
ICH = S // 128       # i chunks (8)
F32 = mybir.dt.float32
F16 = mybir.dt.float16
WU_N = 10            # PE warm-up matmuls before the score streams

# packed layouts (f16)
# pkD [128, 6336]: weights cols k*32 + (0:16)=W_h, (16:32)=W_d, then
#                  dep at cols 192 + jc*3072 + k*512 + s'
# pkS [80, 2080]:  one-hot selectors rows 0:48 cols 0:2048; transpose
#                  identity rows 64:80 cols 2048:2064; bias col 2064
PKA_D = 192          # dep column base in pkD
PKA_N = PKA_D + 2 * KCH * 512
PKS_ID = LH * 128    # 2048
PKS_B = PKS_ID + LH  # 2064
PKS_N = 2080

# knobs for test harness
TRACE = False
TRACE_CORES = None
LAST_RESULTS = None

_CACHE = {}


def _build():
    nc = bacc.Bacc("TRN2", target_bir_lowering=False, debug=False)
    # inputs pre-rearranged on host: [partition, jc, k, s'] where
    # d = k*128 + p contracts and j (or i) = jc*512 + s'
    headT = nc.dram_tensor("headT", [128, 2, KCH, 512], F16,
                           kind="ExternalInput")
    pkdd = nc.dram_tensor("pkD", [128, PKA_N], F16, kind="ExternalInput")
    pksd = nc.dram_tensor("pkS", [80, PKS_N], F16, kind="ExternalInput")
    # [l, p, c, j]: row i = c*128 + p of label l lives at out[l, p, c, :]
    out = nc.dram_tensor("out", [LH, 128, ICH, S], F16, kind="ExternalOutput")
    out_v = out[:]

    headT_f = headT[:]

    with TileContext(nc) as tc:
        with (
            tc.tile_pool(name="const", bufs=1) as cpool,
            tc.tile_pool(name="outp", bufs=4) as opool,
            tc.tile_pool(name="bcast", bufs=16) as bpool,
            tc.tile_pool(name="psum_sc", bufs=2, space="PSUM") as psc,
            tc.tile_pool(name="psum_tp", bufs=2, space="PSUM") as ptp,
            tc.tile_pool(name="psum_bc", bufs=4, space="PSUM") as pbc,
        ):
            pk_d = cpool.tile([128, PKA_N], F16)
            headT_sb = cpool.tile([128, 2, KCH, 512], F16)
            pk_sel = cpool.tile([80, PKS_N], F16)
            h_lT = cpool.tile([128, S], F16)     # h scores [l, i] @ parts 64:80
            h_all = cpool.tile([128, ICH, LH], F32)  # h scores, [i, l] layout
            d_sb = cpool.tile([48, S], F16)      # d+bias: jc0 @ 0:16, jc1 @ 32:48
            wu_w = cpool.tile([128, LH], F16)    # PE warm-up operands
            wu_x = cpool.tile([128, 512], F16)

            def w_h(k):
                return pk_d[:, k * 32:k * 32 + LH]

            def w_d(k):
                return pk_d[:, k * 32 + LH:k * 32 + 2 * LH]

            def dep_v(jc, k):
                c0 = PKA_D + jc * KCH * 512 + k * 512
                return pk_d[:, c0:c0 + 512]

            def sel_v(jc, lb):
                p0 = 32 * jc
                return pk_sel[p0:p0 + LH, lb * 128:(lb + 1) * 128]

            id_v = pk_sel[64:64 + LH, PKS_ID:PKS_ID + LH]
            b_col = pk_sel[0:48, PKS_B:PKS_B + 1]

            # Warm-up operand memsets first so DVE clears them at t~0 and
            # the PE warm-up chain starts immediately.
            nc.vector.memset(wu_w[:], 0.0)
            nc.vector.memset(wu_x[:], 0.0)

            # --- input staging -------------------------------------------
            # HWDGE descriptor generation is ONE shared TPB-level engine:
            # every transfer's generation serializes (~2 us each) no
            # matter which ring it sits on, and a single queue's drain
            # already hits ~420 GB/s. So: exactly TWO input transfers,
            # both on the sync ring (weights+dep pack first, full head
            # second - each gen hides under the previous drain), and the
            # scalar ring stays empty so ACT goes straight to compute.
            # The selector pack rides SWDGE as ONE medium transfer (many
            # small SWDGE transfers spray descriptor-ring traffic that
            # stalls the HWDGE SDMA engines).
            nc.sync.dma_start(out=pk_d[:], in_=pkdd[:])
            nc.gpsimd.dma_start(out=pk_sel[:], in_=pksd[:])
            # head j-halves as separate transfers: the h-jc0 score stream
            # starts ~2.5us earlier, and the dep pack's tail packets are
            # not interleaved with head packets (same-ring FIFO)
            nc.sync.dma_start(out=headT_sb[:, 0], in_=headT_f[:, 0])
            nc.sync.dma_start(out=headT_sb[:, 1], in_=headT_f[:, 1])

            # PE warm-up (builds HAM clock duty while inputs stream in;
            # sized to end about when dep lands). Lives in the transpose
            # pool so it never blocks the score psums.
            wu_ps = ptp.tile([128, 512], F32, name="wu", tag="tp")
            for _ in range(WU_N):
                nc.tensor.matmul(wu_ps[0:LH, :], wu_w[:], wu_x[:],
                                 start=True, stop=True)

            # d scores: two concurrent column-group streams (jc0 @ group 0,
            # jc1 @ group 32), issue-interleaved so the array pipelines the
            # LdWeights of one group under the matmul of the other.
            sc_a = psc.tile([128, 512], F32, name="sc_a", tag="score")
            sc_b = psc.tile([128, 512], F32, name="sc_b", tag="score")
            for k in range(KCH):
                nc.tensor.matmul(
                    sc_a[0:LH, :], w_d(k), dep_v(0, k),
                    start=(k == 0), stop=(k == KCH - 1),
                    tile_position=(0, 0),
                )
                nc.tensor.matmul(
                    sc_b[32:32 + LH, :], w_d(k), dep_v(1, k),
                    start=(k == 0), stop=(k == KCH - 1),
                    tile_position=(0, 32),
                )

            # d evacuation (+bias) on ACT (fastest PSUM reader), f16 out
            nc.scalar.add(d_sb[0:LH, 0:512], sc_a[0:LH, :], b_col[0:LH, :])
            nc.scalar.add(d_sb[32:32 + LH, 512:1024],
                          sc_b[32:32 + LH, :], b_col[32:32 + LH, :])

            # h j-half 0 @ group 64 (needs head-jc0 only)
            sc_c = psc.tile([128, 512], F32, name="sc_c", tag="score")
            for k in range(KCH):
                nc.tensor.matmul(
                    sc_c[64:64 + LH, :], w_h(k), headT_sb[:, 0, k, :],
                    start=(k == 0), stop=(k == KCH - 1),
                    tile_position=(0, 64),
                )
            # h evac on DVE: ACT is serially busy with the d evacuations
            # and dbc0 copies right now; DVE is idle
            nc.vector.tensor_copy(out=h_lT[64:64 + LH, 0:512],
                                  in_=sc_c[64:64 + LH, :])

            dbcs = {}

            def bcast(lb):
                # replicate d row lb across 128 partitions: one-hot selector
                # matmuls (f16 exact). Result evacuated to a persistent f16
                # SBUF tile so the adds run in DVE 4x mode.
                dbc = bpool.tile([128, S], F16, name="dbc", tag="dbc")
                for jc in range(2):
                    bc_ps = pbc.tile([128, 512], F32, name="bc", tag="bc")
                    nc.tensor.matmul(
                        bc_ps[:], sel_v(jc, lb),
                        d_sb[32 * jc:32 * jc + LH,
                             jc * 512:(jc + 1) * 512],
                        start=True, stop=True,
                    )
                    nc.scalar.copy(dbc[:, jc * 512:(jc + 1) * 512], bc_ps[:])
                dbcs[lb] = dbc

            # first broadcast as soon as d_sb exists (bcast(1) comes after
            # label 0's first adds so it never sits ahead of them in the
            # in-order ACT queue)
            bcast(0)

            # h -> [i, l] layout via PE transposes of [16, 128] blocks
            def h_transpose(ic):
                loc = ic * 128
                tp = ptp.tile([128, LH], F16, name="tp", tag="tp")
                nc.tensor.transpose(
                    tp[:], h_lT[64:64 + LH, loc:loc + 128], id_v)
                nc.vector.tensor_copy(out=h_all[:, ic, :], in_=tp[:])

            for ic in range(4):
                h_transpose(ic)

            def add_one(ot, lb, ic, on_dve):
                scal = h_all[:, ic, lb:lb + 1]
                if on_dve:
                    nc.vector.tensor_scalar_add(ot[:, ic, :], dbcs[lb][:],
                                                scal)
                else:
                    nc.scalar.add(ot[:, ic, :], dbcs[lb][:], scal)

            # label 0, first half: launches the output stream before the
            # h j-half-1 path resolves (all-DVE: ACT is still busy with
            # the dbc0 evacuation at this point)
            ot0 = opool.tile([128, ICH, S], F16, name="ot", tag="ot")
            for ic in range(4):
                add_one(ot0, 0, ic, on_dve=True)
            nc.sync.dma_start(out=out_v[0, :, 0:4, :], in_=ot0[:, 0:4, :])
            bcast(1)

            # h j-half 1, also @ group 64 (its bank is sc_b's, its column
            # group reopens once sc_c is evacuated; PE is free by then)
            sc_d = psc.tile([128, 512], F32, name="sc_d", tag="score")
            for k in range(KCH):
                nc.tensor.matmul(
                    sc_d[64:64 + LH, :], w_h(k), headT_sb[:, 1, k, :],
                    start=(k == 0), stop=(k == KCH - 1),
                    tile_position=(0, 64),
                )
            nc.vector.tensor_copy(out=h_lT[64:64 + LH, 512:1024],
                                  in_=sc_d[64:64 + LH, :])
            for ic in range(4, ICH):
                h_transpose(ic)

            # label 0, second half
            for ic in range(4, ICH):
                add_one(ot0, 0, ic, on_dve=(ic < 6))
            nc.sync.dma_start(out=out_v[0, :, 4:8, :], in_=ot0[:, 4:8, :])
            bcast(2)

            # --- steady output loop --------------------------------------
            for lb in range(1, LH):
                ot = opool.tile([128, ICH, S], F16, name="ot", tag="ot")
                for ic in range(ICH):
                    add_one(ot, lb, ic, on_dve=(ic < 6))
                nc.sync.dma_start(out=out_v[lb, :, :, :], in_=ot[:])
                # broadcasts emitted AFTER each label's adds: on the
                # in-order ACT queue the dbc copies must sit behind this
                # label's adds, or every label gates on the next label's
                # broadcast evacuation
                if lb + 2 < LH:
                    bcast(lb + 2)
    nc.compile()
    return nc


def kernel(head, dep, label_W, label_b):
    global LAST_RESULTS
    head = np.asarray(head, dtype=np.float32)
    dep = np.asarray(dep, dtype=np.float32)
    label_W = np.asarray(label_W, dtype=np.float32)
    label_b = np.asarray(label_b, dtype=np.float32)

    def pack_inp(x):  # [S, D] f32 -> [128, 2, KCH, 512] f16, d = k*128+p
        xT = np.ascontiguousarray(x.T).astype(np.float16)   # [D, S]
        return np.ascontiguousarray(
            xT.reshape(KCH, 128, 2, 512).transpose(1, 2, 0, 3))

    headP = [pack_inp(head[b]) for b in range(B)]
    depP = [pack_inp(dep[b]) for b in range(B)]
    whT = label_W[:, :D].T.astype(np.float16)   # [D, L]
    wdT = label_W[:, D:].T.astype(np.float16)   # [D, L]

    in_maps = []
    for c in range(NCORES):
        b, lh = divmod(c, 2)
        ls = slice(lh * LH, (lh + 1) * LH)
        pack_d = np.zeros((128, PKA_N), dtype=np.float16)
        # weights: pack_d[p, k*32 + c] = W[d = k*128 + p, label c]
        for k in range(KCH):
            pack_d[:, k * 32:k * 32 + LH] = whT[k * 128:(k + 1) * 128, ls]
            pack_d[:, k * 32 + LH:k * 32 + 2 * LH] = \
                wdT[k * 128:(k + 1) * 128, ls]
        # dep at cols 192+: [p, jc, k, s']
        pack_d[:, PKA_D:] = depP[b].reshape(128, 2 * KCH * 512)
        pack_s = np.zeros((80, PKS_N), dtype=np.float16)
        # one-hot selectors at partition groups 0 and 32
        for lb in range(LH):
            pack_s[lb, lb * 128:(lb + 1) * 128] = 1.0
            pack_s[32 + lb, lb * 128:(lb + 1) * 128] = 1.0
        # transpose identity at partition group 64
        pack_s[64:64 + LH, PKS_ID:PKS_ID + LH] = np.eye(LH, dtype=np.float16)
        # bias column at partition groups 0 and 32
        pack_s[0:LH, PKS_B] = label_b[ls]
        pack_s[32:48, PKS_B] = label_b[ls]
        in_maps.append({
            "headT": headP[b],
            "pkD": pack_d,
            "pkS": pack_s,
        })

    if "nc" not in _CACHE:
        _CACHE["nc"] = _build()
    nc = _CACHE["nc"]

    res = run_bass_kernel_spmd(nc, in_maps, core_ids=list(range(NCORES)),
                               trace=TRACE, trace_cores=TRACE_CORES)
    LAST_RESULTS = res

    out = np.empty((B, L, S, S), dtype=np.float32)
    for c in range(NCORES):
        b, lh = divmod(c, 2)
        # device layout [l, p, c, j] with i = c*128 + p -> [l, i, j]
        o = np.asarray(res.results[c]["out"])  # [16, 128, 8, 1024] f16
        o = o.transpose(0, 2, 1, 3).reshape(LH, S, S)
        out[b, lh * LH:(lh + 1) * LH] = o.astype(np.float32)
    return out
